# revision 14
# baseline (speedup 1.0000x reference)
"""Trainium2 Bass kernel for nn_CNNNer (sparse band biaffine NER scorer).

Math collapse (everything after the GELU stage is linear):
  head = gelu(state@Wh+bh) ++ [1]          (features 0..200, 200 is the 1)
  tail = gelu(state@Wt+bt) ++ [1]
  band[n,r,k] = head[n]^T U''_k tail[m],  m = n+r-64
      with U''_k = U_k + e_200 Wtp[k,:] + Whp[k,:]^T e_200^T
  scores'[n,r,t] = head[n]^T UW_t tail[m],  UW_t = sum_k Wd[k,t] U''_k
      (UW precomputed on host, [9,201,201]); scores = scores' + bd.
  Query/key padding masks zero whole band entries independently, so ALL
  masking is applied on host after the gather (masked entries := bd).

Device work per core (8 cores; core = (batch b, query quarter), 256 queries):
  1. headT/tailT = gelu MLPs, transposed ([feature, position]), bf16.
  2. step A: UhT_t[j, x] = sum_i UW[t,i,j] headT[i,x]        (9 tags)
  3. step B: S_t[x, m]  = sum_j UhT_t[j, x] tailT[j, m]      (two 128x256
     windows per core; band diagonals extracted on host)

Feature dim F=201 is chunked (104, 97); the constant-1 feature sits at
local partition 96 of chunk 2 (96 is engine-alignment-legal for memset).

DMA design (v3): all transfers are 128-partition DMAs with FLAT per-
partition-contiguous layouts — descriptor-size probe showed bigger runs
are faster (400B/ns at 6KB vs 190B/ns at 1.5KB) and spread across all 16
engines; only <128-partition DMAs pin to one engine. UW is zero-padded
to 128 partitions. Loads split across sync/scalar (+gpsimd for the last
uw third) in critical-path order. PSUM pools are scoped per phase, and
dummy warm-up matmuls ramp the PE p-state during the initial DMA wait.
"""

import os

import numpy as np

B, N, HID = 2, 1024, 768
BSZ = 200
W = 64
TAGS = 9
F = BSZ + 1  # 201 features incl the ones column
NQ = 256  # queries per core
NW = NQ + 2 * W  # 384 window positions per core
R = 2 * W + 1  # 129 band offsets
NCORES = 8
C1 = 104  # feature chunk 1: i/j = 0..103
C2 = 97  # feature chunk 2: i/j = 104..200 (local 96 = ones row)
C2G = 96  # gelu rows of chunk 2 (features 104..199)
JP = 204  # j-row stride inside uw rows
HT = 6  # 768/128 contraction chunks
TS = 3  # tags per output store
NWARM = 15  # PE p-state warm-up matmuls

_cache: dict = {}


def io_dt_name():
    return os.environ.get("BASSK_IO_DT", "bf16")


def _build_nc():
    import concourse.mybir as mybir
    import concourse.tile as tile
    from concourse import bacc

    dt = mybir.dt
    f32 = dt.float32
    ion = io_dt_name()
    io = {"f32": f32, "f32r": dt.float32r, "bf16": dt.bfloat16}[ion]

    nc = bacc.Bacc(
        "TRN2", target_bir_lowering=False, debug=False, enable_asserts=False
    )
    # flat per-partition layouts (see module docstring)
    xd = nc.dram_tensor("xd", [128, HT * NW], io, kind="ExternalInput").ap()
    wd = nc.dram_tensor("wd", [128, 2400], io, kind="ExternalInput").ap()
    uwd = nc.dram_tensor("uwd", [128, 18 * JP], io, kind="ExternalInput").ap()
    # bias cols: bh[0:104], bt[0:104], bh[104:200]+pad, bt[104:200]+pad
    bias4 = nc.dram_tensor("bias4", [128, 4], f32, kind="ExternalInput").ap()
    sout = nc.dram_tensor("sout", [2, 128, TAGS, NQ], io, kind="ExternalOutput").ap()

    gelu = {
        "gelu": mybir.ActivationFunctionType.Gelu,
        "identity": mybir.ActivationFunctionType.Identity,
    }[os.environ.get("BASSK_ACT", "gelu")]

    with tile.TileContext(nc) as tc:
        with tc.tile_pool(name="sb", bufs=1) as sb:
            x_sb = sb.tile([128, HT * NW], io)
            w_sb = sb.tile([128, 2400], io)
            uw_sb = sb.tile([128, 18 * JP], io)
            b_sb = sb.tile([128, 4], f32)
            scr = sb.tile([128, NQ], io)

            def xsl(ht, c0, ncols):
                off = ht * NW + c0
                return x_sb[:, off : off + ncols]

            def wsl(off, n):
                return w_sb[:, off : off + n]

            def usl(t, c, j0, jw):
                part = C1 if c == 0 else C2
                off = (2 * t + c) * JP + j0
                return uw_sb[0:part, off : off + jw]

            # ---- loads: critical-path order across queues ----
            # w's first chunk leads on sync (scalar's queue starts later:
            # ACT_TABLE_LOAD runs first on that engine); gpsimd only takes
            # the tiny bias so the engine stays free for PSUM copies.
            H3 = HT * NW // 2
            nc.sync.dma_start(out=x_sb[:, 0:H3], in_=xd[:, 0:H3])
            nc.scalar.dma_start(out=w_sb[:, 0:400], in_=wd[:, 0:400])
            nc.scalar.dma_start(out=w_sb[:, 400:1200], in_=wd[:, 400:1200])
            nc.sync.dma_start(out=x_sb[:, H3 : 2 * H3], in_=xd[:, H3 : 2 * H3])
            nc.scalar.dma_start(out=w_sb[:, 1200:2400], in_=wd[:, 1200:2400])
            nc.gpsimd.dma_start(out=b_sb, in_=bias4)
            U3 = 6 * JP
            nc.sync.dma_start(out=uw_sb[:, 0:U3], in_=uwd[:, 0:U3])
            nc.scalar.dma_start(out=uw_sb[:, U3 : 2 * U3], in_=uwd[:, U3 : 2 * U3])
            nc.gpsimd.dma_start(
                out=uw_sb[:, 2 * U3 : 3 * U3], in_=uwd[:, 2 * U3 : 3 * U3]
            )

            headT1 = sb.tile([C1, NQ], io)
            headT2 = sb.tile([C2, NQ], io)
            tailT1 = sb.tile([C1, NW], io)
            tailT2 = sb.tile([C2, NW], io)
            uh1 = sb.tile([C1, TAGS, NQ], io)
            uh2 = sb.tile([C2, TAGS, NQ], io)
            s_sb0 = sb.tile([128, TAGS, NQ], io)
            s_sb1 = sb.tile([128, TAGS, NQ], io)

            # ---- PE p-state warm-up during the DMA wait ----
            nc.vector.memset(scr, 0.0)
            with tc.tile_pool(name="ps_w", bufs=1, space="PSUM") as ps_w:
                pw = ps_w.tile([128, NQ], f32, tag="pw")
                for i in range(NWARM):
                    nc.tensor.matmul(
                        pw, scr[:, 0:128], scr,
                        start=(i == 0), stop=(i == NWARM - 1),
                    )

            # ---- MLPs: o = gelu(W^T x + b), transposed; ones via memset ----
            with tc.tile_pool(name="ps_mlp", bufs=2, space="PSUM") as ps_mlp:
                for woff, c0, ncols, o1, o2, bc in (
                    (0, W, NQ, headT1, headT2, 0),
                    (200, 0, NW, tailT1, tailT2, 1),
                ):
                    pm1 = ps_mlp.tile([C1, ncols], f32, tag="pm1")
                    pm2 = ps_mlp.tile([C2G, ncols], f32, tag="pm2")
                    for ht in range(HT):
                        xa = xsl(ht, c0, ncols)
                        nc.tensor.matmul(
                            pm1, wsl(ht * 400 + woff, C1), xa,
                            start=(ht == 0), stop=(ht == HT - 1),
                        )
                        nc.tensor.matmul(
                            pm2, wsl(ht * 400 + woff + C1, C2G), xa,
                            start=(ht == 0), stop=(ht == HT - 1),
                        )
                    nc.scalar.activation(
                        out=o1[0:C1, :], in_=pm1, func=gelu,
                        bias=b_sb[0:C1, bc : bc + 1],
                    )
                    nc.scalar.activation(
                        out=o2[0:C2G, :], in_=pm2, func=gelu,
                        bias=b_sb[0:C2G, bc + 2 : bc + 3],
                    )
                    nc.vector.memset(o2[C2G:C2, :], 1.0)

            with (
                tc.tile_pool(name="ps_a", bufs=3, space="PSUM") as ps_a,
                tc.tile_pool(name="ps_s", bufs=5, space="PSUM") as ps_s,
            ):
                # ---- step A: UhT_t[j, x] = sum_i UW[t,i,j] headT[i,x] ----
                for t in range(TAGS):
                    for j0, jw, uh in ((0, C1, uh1), (C1, C2, uh2)):
                        pa = ps_a.tile([jw, NQ], f32, tag="pa")
                        nc.tensor.matmul(
                            pa, usl(t, 0, j0, jw), headT1, start=True, stop=False
                        )
                        nc.tensor.matmul(
                            pa, usl(t, 1, j0, jw), headT2, start=False, stop=True
                        )
                        nc.any.tensor_copy(uh[:, t, :], pa)

                # ---- step B: S_t[x, m] = sum_j UhT_t[j, x] tailT[j, m] ----
                # tag-outer/qc-inner so the final stores land on BOTH
                # queues in parallel instead of bunching on one
                qs = (nc.sync, nc.scalar)
                for t in range(TAGS):
                    for qc in range(2):
                        s_sb = (s_sb0, s_sb1)[qc]
                        pS = ps_s.tile([128, NQ], f32, tag="ps")
                        nc.tensor.matmul(
                            pS, uh1[:, t, qc * 128 : qc * 128 + 128],
                            tailT1[:, qc * 128 : qc * 128 + NQ],
                            start=True, stop=False,
                        )
                        nc.tensor.matmul(
                            pS, uh2[:, t, qc * 128 : qc * 128 + 128],
                            tailT2[:, qc * 128 : qc * 128 + NQ],
                            start=False, stop=True,
                        )
                        nc.any.tensor_copy(s_sb[:, t, :], pS)
                        # triples early; singles for the last tags so the
                        # final store transfer (the kernel tail) is small
                        if t in (2, 5):
                            qs[qc].dma_start(
                                out=sout[qc, :, t - 2 : t + 1, :],
                                in_=s_sb[:, t - 2 : t + 1, :],
                            )
                        elif t >= 6:
                            qs[qc].dma_start(
                                out=sout[qc, :, t : t + 1, :],
                                in_=s_sb[:, t : t + 1, :],
                            )

    nc.compile()
    return nc


def _np_io_dt():
    if io_dt_name() == "bf16":
        import ml_dtypes

        return ml_dtypes.bfloat16
    return np.float32


def _get_nc():
    key = "nc-" + io_dt_name()
    if key not in _cache:
        _cache[key] = _build_nc()
    return _cache[key]


def _install_ntff_hook():
    """Profiling-only (BASSK_TRACE=1): provide antenv.axon_hooks if the
    image lacks it, wired to the libaxon NTFF capture via ctypes."""
    import sys
    import types

    try:
        from antenv.axon_hooks import get_axon_ntff_profile_hook  # noqa: F401

        return
    except ImportError:
        pass
    from trn_agent_boot.trn_boot import _ntff_profile_via_ctypes

    hook = _ntff_profile_via_ctypes("/opt/axon/libaxon_pjrt.so")
    mod = types.ModuleType("antenv.axon_hooks")
    mod._hook = hook
    mod.get_axon_ntff_profile_hook = lambda: mod._hook
    mod.set_axon_ntff_profile_hook = lambda h: setattr(mod, "_hook", h)
    sys.modules["antenv.axon_hooks"] = mod


def _host_prep(state, Wh, bh, Wt, bt, U, Wcat, Wd):
    """Fold U/Wcat/Wd into UW[9,201,201]; pack flat per-core inputs."""
    iodt = _np_io_dt()

    Whp = Wcat[:, :F]
    Wtp = Wcat[:, F:]
    U2 = U.astype(np.float64).copy()
    U2[:, F - 1, :] += Wtp  # head ones-row picks up the tail term
    U2[:, :, F - 1] += Whp  # tail ones-col picks up the head term
    UW = np.einsum("kt,kij->tij", Wd.astype(np.float64), U2).astype(np.float32)

    # uwd[p, (2t+c)*JP + j] = UW[t, ioff_c + p, j]
    uwd = np.zeros((128, 18 * JP), np.float32)
    for t in range(TAGS):
        for c, (ioff, part) in enumerate(((0, C1), (C1, C2))):
            off = (2 * t + c) * JP
            uwd[0:part, off : off + F] = UW[t, ioff : ioff + part, :]
    uwd = np.ascontiguousarray(uwd.astype(iodt))

    # wd[p, ht*400 + {0,200}] = Wh/Wt[ht*128+p, :]
    wd = np.zeros((128, 2400), np.float32)
    for ht in range(HT):
        wd[:, ht * 400 : ht * 400 + 200] = Wh[ht * 128 : (ht + 1) * 128, :]
        wd[:, ht * 400 + 200 : ht * 400 + 400] = Wt[ht * 128 : (ht + 1) * 128, :]
    wd = np.ascontiguousarray(wd.astype(iodt))

    bias4 = np.zeros((128, 4), np.float32)
    bias4[0:C1, 0] = bh[0:C1]
    bias4[0:C1, 1] = bt[0:C1]
    bias4[0:C2G, 2] = bh[C1:BSZ]
    bias4[0:C2G, 3] = bt[C1:BSZ]

    in_maps = []
    for b in range(B):
        for qi in range(N // NQ):
            q0 = qi * NQ
            lo = q0 - W
            xw = np.zeros((NW, HID), np.float32)
            s, e = max(lo, 0), min(q0 + NQ + W, N)
            xw[s - lo : e - lo] = state[b, s:e]
            # xd[p, ht*NW + c] = xw[c, ht*128+p]
            xflat = (
                xw.T.reshape(HT, 128, NW).transpose(1, 0, 2).reshape(128, HT * NW)
            )
            in_maps.append(
                {
                    "xd": np.ascontiguousarray(xflat.astype(iodt)),
                    "wd": wd,
                    "uwd": uwd,
                    "bias4": bias4,
                }
            )
    return in_maps


def _assemble(outs, bd, lengths):
    """outs: NCORES arrays [2, 128, TAGS, NQ] -> scores [B, N, R, TAGS]."""
    scores = np.empty((B, N, R, TAGS), np.float32)
    mi = np.arange(128)[:, None] + np.arange(R)[None, :]  # [128, R]
    for c, S in enumerate(outs):
        b, qi = divmod(c, N // NQ)
        for qc in range(2):
            g = np.take_along_axis(
                S[qc].astype(np.float32), mi[:, None, :], axis=2
            )  # [128, TAGS, R]
            scores[b, qi * NQ + qc * 128 : qi * NQ + (qc + 1) * 128] = g.transpose(
                0, 2, 1
            )
    bdf = bd.astype(np.float32)
    scores += bdf[None, None, None, :]
    # host-side pad mask: masked entries equal bd exactly (0 @ Wd + bd)
    j_idx = np.arange(N)[:, None] + np.arange(R)[None, :] - W  # [N, R]
    in_range = (j_idx >= 0) & (j_idx < N)
    for b in range(B):
        key_ok = in_range & (j_idx < lengths[b])
        q_ok = np.arange(N) < lengths[b]
        pad = ~(key_ok & q_ok[:, None])  # [N, R]
        scores[b][pad] = bdf
    return np.where(np.isfinite(scores), scores, 0.0).astype(np.float32)


def kernel(**inputs):
    state = np.asarray(inputs["state"], np.float32)
    lengths = np.asarray(inputs["lengths"]).astype(np.int64)
    Wh = np.ascontiguousarray(np.asarray(inputs["Wh"], np.float32))
    bh = np.asarray(inputs["bh"], np.float32)
    Wt = np.ascontiguousarray(np.asarray(inputs["Wt"], np.float32))
    bt = np.asarray(inputs["bt"], np.float32)
    U = np.asarray(inputs["U"], np.float32)
    Wcat = np.asarray(inputs["Wcat"], np.float32)
    Wd = np.asarray(inputs["Wd"], np.float32)
    bd = np.asarray(inputs["bd"], np.float32)

    in_maps = _host_prep(state, Wh, bh, Wt, bt, U, Wcat, Wd)
    nc = _get_nc()

    if os.environ.get("BASSK_SIM"):
        from concourse.bass_interp import CoreSim

        outs = []
        for im in in_maps[: int(os.environ.get("BASSK_SIM_N", len(in_maps)))]:
            sim = CoreSim(nc, trace=False)
            for k, v in im.items():
                sim.tensor(k)[:] = v
            sim.simulate()
            outs.append(sim.tensor("sout").copy())
        while len(outs) < NCORES:
            outs.append(outs[-1])
    else:
        trace = bool(os.environ.get("BASSK_TRACE"))
        if trace:
            _install_ntff_hook()
        from concourse.bass_utils import run_bass_kernel_spmd

        try:
            res = run_bass_kernel_spmd(
                nc, in_maps, core_ids=list(range(NCORES)), trace=trace
            )
        except Exception:
            # transient NRT/device hiccups recover on a fresh attempt
            import time

            time.sleep(2.0)
            res = run_bass_kernel_spmd(
                nc, in_maps, core_ids=list(range(NCORES)), trace=trace
            )
        _cache["last_result"] = res
        outs = [r["sout"] for r in res.results]

    return _assemble(outs, bd, lengths)
